# revision 20
# baseline (speedup 1.0000x reference)
"""Multi-head attention (B=4, S=2048, E=768, H=8, D=96) on 8 Trainium2 cores.

Sharding: core c -> (batch b = c//2, head-group hg = c%2 of 4 heads).
Each core computes Q/K/V projections for its 4 heads over the full sequence
of its batch, full attention for those heads, and a partial output
projection (row-split Wo).  The two cores of a batch produce partial
outputs that are summed on the host during unsharding (tensor-parallel
reduce).

On-chip layout notes:
  - All matmul operands are bf16 (1 cycle/row on PE; fp8 is 2x faster but
    its ~3.6% per-element noise flows 1:1 to the output (attention's own
    output is sqrt(N)-suppressed) and blows the 2e-2 gate).
  - K/Q projections are PACKED: the 4 heads' 384 q^T/k^T rows fill 3 full
    128-row tiles (no 96->128 padding waste on the PE).  Head->row
    permutation: tile t holds head t's 96 dims at partitions 0:96 and head
    3's dims 32t:32t+32 at partitions 96:128, so heads 0-2 are usable
    in place (base partition 0) and only head 3 needs a rebuild into a
    [96, seq] tile via three cheap SBUF->SBUF 32-row copies (DVE 4x mode).
  - Attention scores are computed transposed, S^T[k, q] = K^T.T @ Q^T;
    softmax sums come free by augmenting V with a ones column (row 96 of
    the O^T accumulator is sum_k exp(S)).
  - exp on ACT straight out of PSUM ([128,1024] per key-tile pair) with
    1/sqrt(d) folded into the activation scale.
  - Normalization per (head, q-chunk) with NO DRAM round-trip: DVE row
    extract [1,512] -> DVE reciprocal -> Pool partition_broadcast ->
    one fused multiply-copy TT into packed attnT (3 TTs only for head 3).
  - V-projection PSUM->SBUF copies run on ACT (activation Copy);
    out-projection PSUM->SBUF copies run on Pool, spreading elementwise
    work across ACT/DVE/Pool so everything hides under the PE roofline.
  - x arrives in col-blocks [128, 384, 512, 512, 512] so the first
    V-projection matmuls start as early as possible.
"""

import os
import sys

sys.path.insert(0, "/opt/trn_rl_repo")

import numpy as np
import ml_dtypes

import concourse.bacc as bacc
import concourse.bass as bass
import concourse.tile as tile
from concourse import mybir
from concourse.bass_utils import run_bass_kernel_spmd

BF16 = ml_dtypes.bfloat16

EMB = 768
HEADS = 8
HD = 96          # true head dim
HDP = 128        # padded head dim (vaug layout)
SEQ = 2048
B = 4
NCORES = 8
HPC = 4          # heads per core
NT = 3           # packed k/q row tiles (4 heads x 96 = 3 x 128)
SCALING = HD ** -0.5
QC = 512         # query chunk per attention inner loop
NQC = SEQ // QC
NKT = SEQ // 128  # 16 key tiles
NPAIR = NKT // 2
NE = EMB // 128   # 6 e_in tiles
XBLK = [0, 128, 512, 1024, 1536, 2048]  # x col-block boundaries

_NC_CACHE = {}
LAST_RESULT = None  # BassKernelResults of the most recent run (for test.py)


def _build_nc():
    f32 = mybir.dt.float32
    bf = mybir.dt.bfloat16

    nc = bacc.Bacc(trn_type="TRN2", target_bir_lowering=False, debug=False,
                   num_devices=NCORES)

    xT = nc.dram_tensor("xT", [EMB, SEQ], bf, kind="ExternalInput").ap()
    wqT = nc.dram_tensor("wqT", [EMB, NT * 128], bf, kind="ExternalInput").ap()
    wkT = nc.dram_tensor("wkT", [EMB, NT * 128], bf, kind="ExternalInput").ap()
    wvT = nc.dram_tensor("wvT", [EMB, HPC * HD], bf, kind="ExternalInput").ap()
    # packed (permuted like k/q tiles): 384 rows = 3 full partition tiles
    woT = nc.dram_tensor("woT", [NT * 128, EMB], bf, kind="ExternalInput").ap()
    bqp = nc.dram_tensor("bqp", [128, NT], f32, kind="ExternalInput").ap()
    bkp = nc.dram_tensor("bkp", [128, NT], f32, kind="ExternalInput").ap()
    outp = nc.dram_tensor("outp", [SEQ, EMB], f32, kind="ExternalOutput").ap()

    with tile.TileContext(nc) as tc:
        with (
            tc.tile_pool(name="const", bufs=1) as constp,
            tc.tile_pool(name="big", bufs=1) as bigp,
            tc.tile_pool(name="expp", bufs=4) as expp,
            tc.tile_pool(name="rbp", bufs=3) as rbp,
            tc.tile_pool(name="outsb", bufs=3) as outsb,
            tc.tile_pool(name="ps_proj", bufs=2, space="PSUM") as ps_proj,
            tc.tile_pool(name="ps_o", bufs=2, space="PSUM") as ps_o,
            tc.tile_pool(name="ps_pair", bufs=2, space="PSUM") as ps_pair,
        ):
            # ---- loads ----
            # x^T in col-blocks (first ones small so V-proj starts early);
            # two e-tiles per DMA (pair tiles) to halve SP issue serialization
            xt = [[None] * (len(XBLK) - 1) for _ in range(NE)]

            def load_xt_pair(ep, blk):
                lo, hi = XBLK[blk], XBLK[blk + 1]
                t = bigp.tile([128, 2, hi - lo], bf, name=f"xt{ep}_{blk}")
                nc.sync.dma_start(
                    out=t,
                    in_=xT[2 * ep * 128:(2 * ep + 2) * 128, lo:hi]
                    .rearrange("(two p) s -> p two s", two=2))
                xt[2 * ep][blk] = t[:, 0, :]
                xt[2 * ep + 1][blk] = t[:, 1, :]

            def load_xt_block(blk):
                for ep in range(NE // 2):
                    load_xt_pair(ep, blk)

            wv_sb = []
            for e in range(NE):
                t = constp.tile([128, HPC * HD], bf, name=f"wv{e}")
                wv_sb.append(t)
            for ep in range(NE // 2):
                nc.sync.dma_start(out=wv_sb[2 * ep],
                                  in_=wvT[2 * ep * 128:(2 * ep + 1) * 128, :])
                nc.sync.dma_start(out=wv_sb[2 * ep + 1],
                                  in_=wvT[(2 * ep + 1) * 128:(2 * ep + 2) * 128, :])
                load_xt_pair(ep, 0)
            for blk in range(1, len(XBLK) - 1):
                load_xt_block(blk)

            def xt_cols(e, lo, width):
                """Slices of x^T cols [lo, lo+width); must lie in one block."""
                for blk in range(len(XBLK) - 1):
                    if XBLK[blk] <= lo < XBLK[blk + 1]:
                        assert lo + width <= XBLK[blk + 1]
                        off = lo - XBLK[blk]
                        return xt[e][blk][:, off:off + width]
                raise AssertionError

            def kq_subchunks(n):
                """Col-ranges of seq-chunk n within single x blocks."""
                lo, hi = n * QC, (n + 1) * QC
                cuts = [lo] + [c for c in XBLK if lo < c < hi] + [hi]
                return list(zip(cuts[:-1], cuts[1:]))

            wq_sb, wk_sb, wo_sb = [], [], []
            for e in range(NE):
                t = constp.tile([128, NT * 128], bf, name=f"wk{e}")
                nc.sync.dma_start(out=t, in_=wkT[e * 128:(e + 1) * 128, :])
                wk_sb.append(t)
                t = constp.tile([128, NT * 128], bf, name=f"wq{e}")
                nc.sync.dma_start(out=t, in_=wqT[e * 128:(e + 1) * 128, :])
                wq_sb.append(t)
            for t_ in range(NT):
                t = constp.tile([128, EMB], bf, name=f"wo{t_}")
                nc.sync.dma_start(out=t, in_=woT[t_ * 128:(t_ + 1) * 128, :])
                wo_sb.append(t)
            bq_sb = constp.tile([128, NT], f32, name="bq_sb")
            nc.sync.dma_start(out=bq_sb, in_=bqp)
            bk_sb = constp.tile([128, NT], f32, name="bk_sb")
            nc.sync.dma_start(out=bk_sb, in_=bkp)

            # ---- persistent intermediates ----
            vaug = []
            for kt in range(NKT):
                t = bigp.tile([128, HPC * HDP], bf, name=f"vaug{kt}")
                nc.gpsimd.memset(t, 0.0)
                ones_cols = t.rearrange("p (h c) -> p h c", h=HPC)[:, :, HD:HD + 1]
                nc.gpsimd.memset(ones_cols, 1.0)
                vaug.append(t)
            # packed q^T/k^T tiles + head-3 rebuilds
            qTp = [bigp.tile([128, SEQ], bf, name=f"qTp{t_}") for t_ in range(NT)]
            kTp = [bigp.tile([128, SEQ], bf, name=f"kTp{t_}") for t_ in range(NT)]
            q3 = bigp.tile([HD, SEQ], bf, name="q3")
            k3 = bigp.tile([HD, SEQ], bf, name="k3")
            # packed attention output (rows permuted like qTp; Wo permuted to match)
            attnT = [bigp.tile([128, SEQ], bf, name=f"attnT{t_}")
                     for t_ in range(NT)]

            def head_qk(h):
                """(qT, kT) views [96 rows, SEQ] for head h."""
                if h < NT:
                    return qTp[h][0:HD, :], kTp[h][0:HD, :]
                return q3, k3

            f32_ = f32

            # ---- projection emit helpers ----
            def emit_v_chunk(kt):
                psv = ps_proj.tile([128, 512], f32_, tag="ps",
                                   name=f"psv{kt}")
                for e in range(NE):
                    nc.tensor.matmul(psv[:, 0:HPC * HD],
                                     lhsT=xt_cols(e, kt * 128, 128),
                                     rhs=wv_sb[e],
                                     start=(e == 0), stop=(e == NE - 1))
                # fp32 PSUM -> bf16 vaug on ACT (one strided copy, 4 heads)
                nc.scalar.activation(
                    vaug[kt].rearrange("p (h c) -> p h c", h=HPC)[:, :, 0:HD],
                    psv[:, 0:HPC * HD].rearrange("p (h c) -> p h c", h=HPC),
                    mybir.ActivationFunctionType.Copy)

            def emit_kq_chunk(t_, n, which):
                """Packed projection: tile t_ of q^T/k^T, seq-chunk n."""
                w_sb, dst, b_sb = ((wk_sb, kTp, bk_sb) if which == "k"
                                   else (wq_sb, qTp, bq_sb))
                ps = ps_proj.tile([128, 512], f32_, tag="ps",
                                  name=f"ps{which}{t_}_{n}")
                for lo, hi in kq_subchunks(n):
                    psl = slice(lo - n * QC, hi - n * QC)
                    for e in range(NE):
                        nc.tensor.matmul(ps[:, psl],
                                         lhsT=w_sb[e][:, t_ * 128:(t_ + 1) * 128],
                                         rhs=xt_cols(e, lo, hi - lo),
                                         start=(e == 0), stop=(e == NE - 1))
                nc.vector.tensor_scalar_add(dst[t_][:, n * QC:(n + 1) * QC], ps,
                                            b_sb[:, t_:t_ + 1])

            def emit_h3_rebuild(which, t_):
                """Copy packed rows 96:128 of tile t_ into the head-3 tile."""
                src, dst = ((kTp, k3) if which == "k" else (qTp, q3))
                nc.vector.tensor_copy(dst[32 * t_:32 * t_ + 32, :],
                                      src[t_][96:128, :])

            def kq_chunks(t_):
                for n in range(NQC):
                    yield ("k", t_, n)
                for n in range(NQC):
                    yield ("q", t_, n)

            # ---- output projection chunk (one 128-row q tile) ----
            # Split across two 1-bank psums so it can borrow ps_proj slots;
            # PSUM->SBUF copies go on Pool (ACT is busy with exp, DVE with
            # normalization).
            def emit_out_chunk(qm):
                qsl = slice(qm * 128, (qm + 1) * 128)
                psA = ps_proj.tile([128, 512], f32_, tag="ps",
                                   name=f"poA{qm}")
                psB = ps_proj.tile([128, 512], f32_, tag="ps",
                                   name=f"poB{qm}")
                for t in range(NT):
                    nc.tensor.matmul(psA,
                                     lhsT=attnT[t][:, qsl],
                                     rhs=wo_sb[t][:, 0:512],
                                     start=(t == 0), stop=(t == NT - 1))
                for t in range(NT):
                    nc.tensor.matmul(psB[:, 0:256],
                                     lhsT=attnT[t][:, qsl],
                                     rhs=wo_sb[t][:, 512:768],
                                     start=(t == 0), stop=(t == NT - 1))
                out_sb = outsb.tile([128, EMB], f32_, tag="osb",
                                    name=f"osb{qm}")
                nc.vector.tensor_copy(out_sb[:, 0:512], psA)
                nc.vector.tensor_copy(out_sb[:, 512:768], psB[:, 0:256])
                nc.sync.dma_start(out=outp[qm * 128:(qm + 1) * 128, :],
                                  in_=out_sb)

            # ---- attention emit (with interleaved PE filler work) ----
            def emit_attention_block(h, q_lo, w, thunks, idx,
                                     norm_piece_cb=None):
                """One attention block: head h, q-cols [q_lo, q_lo+w).
                thunks are injected into the PE stream spread across pairs.
                norm_piece_cb(i): called after each 128-col normalization
                piece (only used for the final block, to release out-proj
                chunks progressively)."""
                hsl = slice(h * HDP, (h + 1) * HDP)
                qT_h, kT_h = head_qk(h)
                step = NPAIR // max(len(thunks), 1)
                inject_at = {}
                for i, t in enumerate(thunks):
                    pos = min(1 + i * max(step, 1), NPAIR - 1)
                    inject_at.setdefault(pos, []).append(t)
                qsl = slice(q_lo, q_lo + w)
                pso = ps_o.tile([128, w], f32_, tag="pso",
                                name=f"pso{idx}")
                eps = []

                def emit_ss(p):
                    pss = ps_pair.tile([128, 2 * w], f32_, tag="pss",
                                       name=f"pss{idx}_{p}")
                    for j in range(2):
                        nc.tensor.matmul(
                            pss[:, j * w:(j + 1) * w],
                            lhsT=kT_h[:, (2 * p + j) * 128:
                                      (2 * p + j + 1) * 128],
                            rhs=qT_h[:, qsl],
                            start=True, stop=True)
                    ep = expp.tile([128, 2 * w], bf, tag="exp",
                                   name=f"exp{idx}_{p}")
                    nc.scalar.activation(ep, pss,
                                         mybir.ActivationFunctionType.Exp,
                                         scale=SCALING)
                    eps.append(ep)

                def emit_o(p):
                    for j in range(2):
                        kt = 2 * p + j
                        nc.tensor.matmul(
                            pso,
                            lhsT=vaug[kt][:, hsl],
                            rhs=eps[p][:, j * w:(j + 1) * w],
                            start=(kt == 0), stop=(kt == NKT - 1))

                for p in range(NPAIR):
                    emit_ss(p)
                    for t in inject_at.get(p, []):
                        t()
                    if p >= 1:
                        emit_o(p - 1)
                emit_o(NPAIR - 1)

                # ---- normalization: extract sums row, reciprocal,
                # partition-broadcast on Pool, fused mul-copy ----
                def norm_piece(lo, pw, sub):
                    psl = slice(lo, lo + pw)
                    osl = slice(q_lo + lo, q_lo + lo + pw)
                    srow = rbp.tile([1, pw], f32_, tag="srow",
                                    name=f"srow{idx}_{sub}")
                    nc.vector.tensor_copy(srow, pso[HD:HD + 1, psl])
                    rrow = rbp.tile([1, pw], f32_, tag="rrow",
                                    name=f"rrow{idx}_{sub}")
                    nc.vector.reciprocal_approx_fast(out=rrow, in_=srow)
                    rb = rbp.tile([HD, pw], f32_, tag="rb",
                                  name=f"rb{idx}_{sub}")
                    nc.gpsimd.partition_broadcast(rb, rrow)
                    if h < NT:
                        nc.vector.tensor_mul(
                            out=attnT[h][0:HD, osl],
                            in0=pso[0:HD, psl], in1=rb)
                    else:
                        for t_ in range(NT):
                            src = 32 * t_
                            nc.vector.tensor_mul(
                                out=attnT[t_][96:128, osl],
                                in0=pso[src:src + 32, psl],
                                in1=rb[src:src + 32, :])

                if norm_piece_cb is None:
                    norm_piece(0, w, 0)
                else:
                    for i in range(w // 128):
                        norm_piece(i * 128, 128, i)
                        norm_piece_cb(i)

            def emit_attention(h, thunks_for_qc):
                for qc in range(NQC):
                    emit_attention_block(h, qc * QC, QC,
                                         list(thunks_for_qc(qc)),
                                         h * NQC + qc)

            # ---- emission schedule ----
            for kt in range(NKT):
                emit_v_chunk(kt)
            for which, tt, n in kq_chunks(0):
                emit_kq_chunk(tt, n, which)

            def kq_thunks(tnext):
                def f(qc):
                    if tnext >= NT:
                        # all packed tiles done: spread head-3 q/k rebuild
                        # copies (DVE) over this head's q-chunks
                        items = [(w, t_) for w in ("k", "q")
                                 for t_ in range(NT)][2 * qc:2 * qc + 2]
                        return [lambda it=it: emit_h3_rebuild(it[0], it[1])
                                for it in items]
                    # 2 chunks per q-chunk: 8 chunks over 4 qcs
                    items = list(kq_chunks(tnext))[2 * qc:2 * qc + 2]
                    return [lambda it=it: emit_kq_chunk(it[1], it[2], it[0])
                            for it in items]
                return f

            OUT_SCHED = {1: range(0, 4), 2: range(4, 9), 3: range(9, 12)}

            def out_thunks(qc):
                # during h3's q-chunk qc, emit out-proj chunks of completed
                # q rows (front-loaded so the post-loop tail is minimal)
                return [lambda qm=qm: emit_out_chunk(qm)
                        for qm in OUT_SCHED.get(qc, [])]

            for h in range(HPC - 1):
                emit_attention(h, kq_thunks(h + 1))
            h3 = HPC - 1
            for qc in range(NQC - 1):
                emit_attention_block(h3, qc * QC, QC, out_thunks(qc),
                                     h3 * NQC + qc)
            # final q-chunk: normalize in 128-col pieces, each unblocking
            # its out-proj chunk so the tail overlaps
            emit_attention_block(
                h3, (NQC - 1) * QC, QC, out_thunks(NQC - 1),
                h3 * NQC + NQC - 1)
            for qm in range(12, 16):
                emit_out_chunk(qm)

    nc.compile()
    return nc


def _get_nc():
    if "nc" not in _NC_CACHE:
        _NC_CACHE["nc"] = _build_nc()
    return _NC_CACHE["nc"]


# packed row permutation: tile t holds head t dims 0:96 at partitions 0:96
# and head 3 dims 32t:32t+32 at partitions 96:128
def _pack_perm():
    perm = []
    for t in range(NT):
        perm.extend(range(96 * t, 96 * t + 96))
        perm.extend(range(288 + 32 * t, 288 + 32 * t + 32))
    return np.array(perm)


_PERM = _pack_perm()


def _pack_wT(w_rows):
    """[384, 768] head rows -> packed/permuted -> transposed [768, 384]."""
    return np.ascontiguousarray(w_rows[_PERM].T).astype(BF16)


def _pack_bias(b_rows):
    """[384] head bias -> [128, NT] permuted for per-partition add."""
    return np.ascontiguousarray(b_rows[_PERM].reshape(NT, 128).T)


def kernel(x, Wq, bq, Wk, bk, Wv, bv, Wo, bo):
    x = np.asarray(x, np.float32)
    Wq, bq = np.asarray(Wq, np.float32), np.asarray(bq, np.float32)
    Wk, bk = np.asarray(Wk, np.float32), np.asarray(bk, np.float32)
    Wv, bv = np.asarray(Wv, np.float32), np.asarray(bv, np.float32)
    Wo, bo = np.asarray(Wo, np.float32), np.asarray(bo, np.float32)

    nc = _get_nc()

    in_maps = []
    for c in range(NCORES):
        b, hg = divmod(c, 2)
        hs = slice(hg * HPC * HD, (hg + 1) * HPC * HD)
        in_maps.append({
            "xT": np.ascontiguousarray(x[b].T).astype(BF16),
            "wqT": _pack_wT(Wq[hs]),
            "wkT": _pack_wT(Wk[hs]),
            "wvT": np.ascontiguousarray(Wv[hs].T).astype(BF16),
            "woT": np.ascontiguousarray(Wo[:, hs].T[_PERM]).astype(BF16),
            "bqp": _pack_bias(bq[hs]),
            "bkp": _pack_bias(bk[hs]),
        })

    global LAST_RESULT
    trace = bool(int(os.environ.get("KERNEL_TRACE", "0")))
    tmpdir = os.environ.get("KERNEL_TRACE_DIR") or None
    res = run_bass_kernel_spmd(nc, in_maps, list(range(NCORES)), trace=trace,
                               tmpdir=tmpdir)
    LAST_RESULT = res

    out = np.empty((B, SEQ, EMB), np.float32)
    for b in range(B):
        out[b] = res.results[2 * b]["outp"] + res.results[2 * b + 1]["outp"]
    # bv enters each head's output additively (sum of softmax weights is 1),
    # and bo is a plain add: both fold into one constant vector.
    out += Wo @ bv + bo
    return out
